# revision 8
# baseline (speedup 1.0000x reference)
"""Multi-head attention (B=2, S=2048, H=2048, NH=16) on 8 TRN2 NeuronCores.

Sharding: tensor-parallel over heads — 2 heads per core. Each core computes
q/k/v projections for its heads, per-head attention, and a partial output
projection (its heads' columns of Wo); the host sums the 8 partials.

Per-core dataflow (all matmuls bf16 inputs, f32 PSUM accumulation):
  - hT [H, B*S] (hidden transposed, bf16, host-prepared) streamed per batch.
  - QT/KT [hd=128, S] per (b, h): feature-major, from wT chunks (stationary)
    x hT (moving).
  - V [t, hd] token-major per b: from hT chunks (stationary) x wvT (moving).
  - scores transposed: ST[t_chunk=128, q] = (KT chunk).T @ QT -> PSUM;
    exp on ScalarE (scale=1/sqrt(hd), bias=-SHIFT) -> P^T bf16 in SBUF.
  - softmax denom: DVE pair/quad tree over P^T chunks, then ones[128,128]
    matmul (broadcasts the column-sum across all 128 partitions), reciprocal.
  - AV: out_avT[hd, q] += V[tc,hd].T @ P^T[tc] -> normalize on DVE -> aoT bf16.
  - O-proj: out[t_tile, o] += aoT[:, t_tile].T @ woT -> f32 partial to DRAM.
"""

import sys

sys.path.insert(0, "/opt/trn_rl_repo")

from contextlib import ExitStack

import ml_dtypes
import numpy as np

import concourse.bass as bass
import concourse.tile as tile
from concourse import bacc, mybir
from concourse.bass_utils import run_bass_kernel_spmd

B, S, H, NH = 2, 2048, 2048, 16
HD = H // NH          # 128
N_CORES = 8
HPC = NH // N_CORES   # heads per core = 2
HDC = HPC * HD        # head-dims per core = 256
T = B * S             # 4096 tokens
FC = H // 128         # 16 feature chunks
TC = S // 128         # 16 token tiles per batch
SHIFT = 4.0           # fixed exp shift (softmax-invariant, overflow guard)

BF16 = mybir.dt.bfloat16
F32 = mybir.dt.float32
EXP = mybir.ActivationFunctionType.Exp
IDENT = mybir.ActivationFunctionType.Identity
COPY = mybir.ActivationFunctionType.Copy

_CACHE = {}


def build_program(out_dtype=F32):
    nc = bacc.Bacc(
        "TRN2", target_bir_lowering=False, debug=False, num_devices=N_CORES
    )
    hT = nc.dram_tensor("hT", [H, T], BF16, kind="ExternalInput").ap()
    wqT = nc.dram_tensor("wqT", [H, HDC], BF16, kind="ExternalInput").ap()
    wkT = nc.dram_tensor("wkT", [H, HDC], BF16, kind="ExternalInput").ap()
    wvT = nc.dram_tensor("wvT", [H, HDC], BF16, kind="ExternalInput").ap()
    woT = nc.dram_tensor("woT", [HDC, H], BF16, kind="ExternalInput").ap()
    bq = nc.dram_tensor("bq", [HDC], F32, kind="ExternalInput").ap()
    bk = nc.dram_tensor("bk", [HDC], F32, kind="ExternalInput").ap()
    bv = nc.dram_tensor("bv", [1, HDC], F32, kind="ExternalInput").ap()
    out = nc.dram_tensor("out", [T, H], out_dtype, kind="ExternalOutput").ap()

    with tile.TileContext(nc) as tc:
        _kernel(tc, out, hT, wqT, wkT, wvT, woT, bq, bk, bv)
    nc.compile()
    return nc


def _kernel(tc, out, hT, wqT, wkT, wvT, woT, bq, bk, bv):
    nc = tc.nc
    ctx = ExitStack()
    with ctx:
        singles = ctx.enter_context(tc.tile_pool(name="singles", bufs=1))
        persist = ctx.enter_context(tc.tile_pool(name="persist", bufs=1))

        # ---- constants / weights resident in SBUF ----
        ones = singles.tile([128, 128], BF16)
        nc.vector.memset(ones, 1.0)
        neg_shift = singles.tile([128, 1], F32)
        nc.vector.memset(neg_shift, -SHIFT)

        w_sb = {}
        for name, ap in (("q", wqT), ("k", wkT), ("v", wvT)):
            t = singles.tile([128, FC, HDC], BF16, tag=f"w{name}")
            nc.sync.dma_start(out=t, in_=ap.rearrange("(c p) m -> p c m", p=128))
            w_sb[name] = t
        woT_sb = singles.tile([128, HPC, H], BF16)
        nc.sync.dma_start(out=woT_sb, in_=woT.rearrange("(h p) o -> p h o", p=128))
        bq_sb = singles.tile([128, HPC], F32)
        nc.sync.dma_start(out=bq_sb, in_=bq.rearrange("(h p) -> p h", p=128))
        bk_sb = singles.tile([128, HPC], F32)
        nc.sync.dma_start(out=bk_sb, in_=bk.rearrange("(h p) -> p h", p=128))
        # bv broadcast across 128 partitions (stride-0 partition dim)
        bv_sb = singles.tile([128, HDC], F32)
        nc.sync.dma_start(
            out=bv_sb,
            in_=bass.AP(tensor=bv.tensor, offset=bv.offset, ap=[[0, 128], [1, HDC]]),
        )

        # persistent activations
        qt_sb = [[persist.tile([128, S], BF16, tag=f"qt{b}{h}", name=f"qt{b}{h}")
                  for h in range(HPC)] for b in range(B)]
        kt_sb = [[persist.tile([128, S], BF16, tag=f"kt{b}{h}", name=f"kt{b}{h}")
                  for h in range(HPC)] for b in range(B)]
        v_sb = [persist.tile([128, TC, HDC], BF16, tag=f"v{b}", name=f"v{b}")
                for b in range(B)]
        aoT_sb = [[persist.tile([128, S], BF16, tag=f"ao{b}{h}", name=f"ao{b}{h}")
                   for h in range(HPC)] for b in range(B)]

        # ================= phase 1: QKV projections =================
        with tc.tile_pool(name="ht", bufs=1) as ht_pool, \
             tc.tile_pool(name="qk_psum", bufs=2, space="PSUM") as qk_psum, \
             tc.tile_pool(name="v_psum", bufs=4, space="PSUM") as v_psum:
            hT_re = hT.rearrange("(c p) t -> p c t", p=128)
            for b in range(B):
                ht_sb = ht_pool.tile([128, FC, S], BF16, tag="ht")
                for qtr in range(4):
                    nc.sync.dma_start(
                        out=ht_sb[:, 4 * qtr : 4 * qtr + 4, :],
                        in_=hT_re[:, 4 * qtr : 4 * qtr + 4, b * S : (b + 1) * S],
                    )
                for h in range(HPC):
                    for name, dst, bias in (
                        ("q", qt_sb[b][h], bq_sb),
                        ("k", kt_sb[b][h], bk_sb),
                    ):
                        for half in range(2):
                            ps = qk_psum.tile([128, 1024], F32, tag="qk")
                            for fc in range(FC):
                                lhsT = w_sb[name][:, fc, h * HD : (h + 1) * HD]
                                for n in range(2):
                                    t0 = half * 1024 + n * 512
                                    nc.tensor.matmul(
                                        ps[:, n * 512 : (n + 1) * 512],
                                        lhsT,
                                        ht_sb[:, fc, t0 : t0 + 512],
                                        start=(fc == 0),
                                        stop=(fc == FC - 1),
                                    )
                            nc.scalar.activation(
                                dst[:, half * 1024 : (half + 1) * 1024],
                                ps,
                                IDENT,
                                bias=bias[:, h : h + 1],
                            )
                # V token-major: lhsT = hT chunk, rhs = wvT
                for tt in range(TC):
                    ps = v_psum.tile([128, HDC], F32, tag="v")
                    for fc in range(FC):
                        nc.tensor.matmul(
                            ps,
                            ht_sb[:, fc, tt * 128 : (tt + 1) * 128],
                            w_sb["v"][:, fc, :],
                            start=(fc == 0),
                            stop=(fc == FC - 1),
                        )
                    nc.vector.tensor_add(v_sb[b][:, tt, :], ps, bv_sb)

        # ================= phase 2: attention =================
        scale = 1.0 / float(np.sqrt(HD))
        with tc.tile_pool(name="sc_psum", bufs=2, space="PSUM") as sc_psum, \
             tc.tile_pool(name="av_psum", bufs=2, space="PSUM") as av_psum, \
             tc.tile_pool(name="pt", bufs=20) as pt_pool, \
             tc.tile_pool(name="pair", bufs=3) as pair_pool, \
             tc.tile_pool(name="quad", bufs=5) as quad_pool, \
             tc.tile_pool(name="den", bufs=2) as den_pool:
            for b in range(B):
                for h in range(HPC):
                    for qh in range(2):
                        q0 = qh * 1024
                        pts = []
                        av = av_psum.tile([128, 1024], F32, tag="av")
                        for tcx in range(TC):
                            ps = sc_psum.tile([128, 1024], F32, tag="sc")
                            lhsT = kt_sb[b][h][:, tcx * 128 : (tcx + 1) * 128]
                            for n in range(2):
                                nc.tensor.matmul(
                                    ps[:, n * 512 : (n + 1) * 512],
                                    lhsT,
                                    qt_sb[b][h][:, q0 + n * 512 : q0 + (n + 1) * 512],
                                    start=True,
                                    stop=True,
                                )
                            pt = pt_pool.tile([128, 1024], BF16, tag="pt")
                            nc.scalar.activation(pt, ps, EXP, bias=neg_shift, scale=scale)
                            pts.append(pt)
                            for n in range(2):
                                nc.tensor.matmul(
                                    av[:, n * 512 : (n + 1) * 512],
                                    v_sb[b][:, tcx, h * HD : (h + 1) * HD],
                                    pt[:, n * 512 : (n + 1) * 512],
                                    start=(tcx == 0),
                                    stop=(tcx == TC - 1),
                                )
                        # denom: pair/quad tree on DVE, then ones-matmul bcast
                        quads = []
                        for i in range(4):
                            pair0 = pair_pool.tile([128, 1024], BF16, tag="pair")
                            nc.vector.tensor_add(pair0, pts[4 * i], pts[4 * i + 1])
                            pair1 = pair_pool.tile([128, 1024], BF16, tag="pair")
                            nc.vector.tensor_add(pair1, pts[4 * i + 2], pts[4 * i + 3])
                            quad = quad_pool.tile([128, 1024], BF16, tag="quad")
                            nc.vector.tensor_add(quad, pair0, pair1)
                            quads.append(quad)
                        den = sc_psum.tile([128, 1024], F32, tag="sc")
                        for i in range(4):
                            for n in range(2):
                                nc.tensor.matmul(
                                    den[:, n * 512 : (n + 1) * 512],
                                    ones,
                                    quads[i][:, n * 512 : (n + 1) * 512],
                                    start=(i == 0),
                                    stop=(i == 3),
                                )
                        recip = den_pool.tile([128, 1024], F32, tag="recip")
                        nc.vector.reciprocal(recip, den)
                        nc.vector.tensor_mul(
                            aoT_sb[b][h][:, q0 : q0 + 1024], av, recip
                        )

        # ================= phase 3: output projection =================
        with tc.tile_pool(name="o_psum", bufs=8, space="PSUM") as o_psum, \
             tc.tile_pool(name="o_sb", bufs=3) as o_sb_pool:
            for b in range(B):
                for tt in range(TC):
                    o_tile = o_sb_pool.tile([128, H], out.dtype, tag="o")
                    for oq in range(4):
                        ps = o_psum.tile([128, 512], F32, tag="o")
                        for h in range(HPC):
                            nc.tensor.matmul(
                                ps,
                                aoT_sb[b][h][:, tt * 128 : (tt + 1) * 128],
                                woT_sb[:, h, oq * 512 : (oq + 1) * 512],
                                start=(h == 0),
                                stop=(h == HPC - 1),
                            )
                        eng = nc.scalar if oq % 2 == 0 else nc.vector
                        if eng is nc.scalar:
                            nc.scalar.activation(
                                o_tile[:, oq * 512 : (oq + 1) * 512], ps, COPY
                            )
                        else:
                            nc.vector.tensor_copy(
                                o_tile[:, oq * 512 : (oq + 1) * 512], ps
                            )
                    nc.sync.dma_start(
                        out=out[b * S + tt * 128 : b * S + (tt + 1) * 128, :],
                        in_=o_tile,
                    )


def kernel(hidden_state, Wq, bq, Wk, bk, Wv, bv, Wo, bo):
    bf16 = ml_dtypes.bfloat16
    h2 = np.asarray(hidden_state, dtype=np.float32).reshape(T, H)
    hT = np.ascontiguousarray(h2.T).astype(bf16)

    in_maps = []
    for c in range(N_CORES):
        r0 = c * HDC
        in_maps.append({
            "hT": hT,
            "wqT": np.ascontiguousarray(
                np.asarray(Wq, np.float32)[r0 : r0 + HDC, :].T).astype(bf16),
            "wkT": np.ascontiguousarray(
                np.asarray(Wk, np.float32)[r0 : r0 + HDC, :].T).astype(bf16),
            "wvT": np.ascontiguousarray(
                np.asarray(Wv, np.float32)[r0 : r0 + HDC, :].T).astype(bf16),
            "woT": np.ascontiguousarray(
                np.asarray(Wo, np.float32)[:, r0 : r0 + HDC].T).astype(bf16),
            "bq": np.asarray(bq, np.float32)[r0 : r0 + HDC].copy(),
            "bk": np.asarray(bk, np.float32)[r0 : r0 + HDC].copy(),
            "bv": np.asarray(bv, np.float32)[r0 : r0 + HDC].reshape(1, HDC).copy(),
        })

    if "nc" not in _CACHE:
        _CACHE["nc"] = build_program()
    nc = _CACHE["nc"]
    _CACHE["in_maps"] = in_maps

    res = run_bass_kernel_spmd(nc, in_maps, core_ids=list(range(N_CORES)))
    total = np.zeros((T, H), np.float32)
    for r in res.results:
        total += np.asarray(r["out"], np.float32)
    total += np.asarray(bo, np.float32)[None, :]
    return total.reshape(B, S, H)


# revision 12
# speedup vs baseline: 1.2201x; 1.2201x over previous
"""Multi-head attention (B=2, S=2048, H=2048, NH=16) on 8 TRN2 NeuronCores.

Sharding: tensor-parallel over heads — 2 heads per core. Each core computes
q/k/v projections for its heads, per-head attention, and a partial output
projection (its heads' columns of Wo); the host sums the 8 partials.

Per-core dataflow (all matmuls bf16 inputs, f32 PSUM accumulation):
  - hT [H, B*S] (hidden transposed, bf16, host-prepared) streamed per batch.
  - QT/KT [hd=128, S] per (b, h): feature-major, from wT chunks (stationary)
    x hT (moving).
  - V [t, hd] token-major per b: from hT chunks (stationary) x wvT (moving).
  - scores transposed: ST[t_chunk=128, q] = (KT chunk).T @ QT -> PSUM;
    exp on ScalarE (scale=1/sqrt(hd), bias=-SHIFT) -> P^T bf16 in SBUF.
  - softmax denom: DVE pair/quad tree over P^T chunks, then ones[128,128]
    matmul (broadcasts the column-sum across all 128 partitions), reciprocal.
  - AV: out_avT[hd, q] += V[tc,hd].T @ P^T[tc] -> normalize on DVE -> aoT bf16.
  - O-proj: out[t_tile, o] += aoT[:, t_tile].T @ woT -> f32 partial to DRAM.
"""

import sys

sys.path.insert(0, "/opt/trn_rl_repo")

from contextlib import ExitStack

import ml_dtypes
import numpy as np

import concourse.bass as bass
import concourse.tile as tile
from concourse import bacc, mybir
from concourse.bass_utils import run_bass_kernel_spmd

B, S, H, NH = 2, 2048, 2048, 16
HD = H // NH          # 128
N_CORES = 8
HPC = NH // N_CORES   # heads per core = 2
HDC = HPC * HD        # head-dims per core = 256
T = B * S             # 4096 tokens
FC = H // 128         # 16 feature chunks
TC = S // 128         # 16 token tiles per batch
SHIFT = 4.0           # fixed exp shift (softmax-invariant, overflow guard)

BF16 = mybir.dt.bfloat16
F32 = mybir.dt.float32
EXP = mybir.ActivationFunctionType.Exp
IDENT = mybir.ActivationFunctionType.Identity
COPY = mybir.ActivationFunctionType.Copy

_CACHE = {}


def build_program(out_dtype=F32):
    nc = bacc.Bacc(
        "TRN2", target_bir_lowering=False, debug=False, num_devices=N_CORES
    )
    hT = nc.dram_tensor("hT", [H, T], BF16, kind="ExternalInput").ap()
    wqT = nc.dram_tensor("wqT", [H, HDC], BF16, kind="ExternalInput").ap()
    wkT = nc.dram_tensor("wkT", [H, HDC], BF16, kind="ExternalInput").ap()
    wvT = nc.dram_tensor("wvT", [H, HDC], BF16, kind="ExternalInput").ap()
    woT = nc.dram_tensor("woT", [HDC, H], BF16, kind="ExternalInput").ap()
    bq = nc.dram_tensor("bq", [HDC], F32, kind="ExternalInput").ap()
    bk = nc.dram_tensor("bk", [HDC], F32, kind="ExternalInput").ap()
    bv = nc.dram_tensor("bv", [1, HDC], F32, kind="ExternalInput").ap()
    out = nc.dram_tensor("out", [T, H], out_dtype, kind="ExternalOutput").ap()

    with tile.TileContext(nc) as tc:
        _kernel(tc, out, hT, wqT, wkT, wvT, woT, bq, bk, bv)
    nc.compile()
    return nc


def _kernel(tc, out, hT, wqT, wkT, wvT, woT, bq, bk, bv):
    nc = tc.nc
    scale = 1.0 / float(np.sqrt(HD))
    ctx = ExitStack()
    with ctx:
        singles = ctx.enter_context(tc.tile_pool(name="singles", bufs=1))
        persist = ctx.enter_context(tc.tile_pool(name="persist", bufs=1))
        ps_sc = ctx.enter_context(tc.tile_pool(name="ps_sc", bufs=2, space="PSUM"))
        ps_wk = ctx.enter_context(tc.tile_pool(name="ps_wk", bufs=2, space="PSUM"))
        ht_pool = ctx.enter_context(tc.tile_pool(name="ht", bufs=3))
        pt_pool = ctx.enter_context(tc.tile_pool(name="pt", bufs=10))
        pair_pool = ctx.enter_context(tc.tile_pool(name="pair", bufs=2))
        quad_pool = ctx.enter_context(tc.tile_pool(name="quad", bufs=5))
        den_pool = ctx.enter_context(tc.tile_pool(name="den", bufs=2))
        o_sb_pool = ctx.enter_context(tc.tile_pool(name="o_sb", bufs=4))

        # ---- constants / weights resident in SBUF ----
        ones = singles.tile([128, 128], BF16)
        nc.vector.memset(ones, 1.0)
        neg_shift = singles.tile([128, 1], F32)
        nc.vector.memset(neg_shift, -SHIFT)

        w_sb = {}
        for name, ap in (("q", wqT), ("k", wkT), ("v", wvT)):
            t = singles.tile([128, FC, HDC], BF16, tag=f"w{name}", name=f"w{name}")
            nc.sync.dma_start(out=t, in_=ap.rearrange("(c p) m -> p c m", p=128))
            w_sb[name] = t
        woT_sb = singles.tile([128, HPC, H], BF16)
        nc.sync.dma_start(out=woT_sb, in_=woT.rearrange("(h p) o -> p h o", p=128))
        bq_sb = singles.tile([128, HPC], F32)
        nc.sync.dma_start(out=bq_sb, in_=bq.rearrange("(h p) -> p h", p=128))
        bk_sb = singles.tile([128, HPC], F32)
        nc.sync.dma_start(out=bk_sb, in_=bk.rearrange("(h p) -> p h", p=128))
        # bv broadcast to [128, 4, 256] (stride-0 partition and group dims)
        bv4 = singles.tile([128, 4, HDC], F32)
        nc.sync.dma_start(
            out=bv4,
            in_=bass.AP(tensor=bv.tensor, offset=bv.offset,
                        ap=[[0, 128], [0, 4], [1, HDC]]),
        )

        # persistent activations
        qt_sb = [[persist.tile([128, S], BF16, tag=f"qt{b}{h}", name=f"qt{b}{h}")
                  for h in range(HPC)] for b in range(B)]
        kt_sb = [[persist.tile([128, S], BF16, tag=f"kt{b}{h}", name=f"kt{b}{h}")
                  for h in range(HPC)] for b in range(B)]
        v_sb = [persist.tile([128, TC, HDC], BF16, tag=f"v{b}", name=f"v{b}")
                for b in range(B)]
        aoT_sb = [[persist.tile([128, S], BF16, tag=f"ao{b}{h}", name=f"ao{b}{h}")
                   for h in range(HPC)] for b in range(B)]

        hT_re = hT.rearrange("(c p) t -> p c t", p=128)

        def qkv(b):
            for half in range(2):
                # two 512-token quarter tiles of hT for this half
                ht_q = []
                for qx in range(2):
                    t0 = b * S + half * 1024 + qx * 512
                    t = ht_pool.tile([128, FC, 512], BF16, tag="ht",
                                     name=f"ht{b}{half}{qx}")
                    for g in range(2):
                        nc.sync.dma_start(
                            out=t[:, 8 * g : 8 * g + 8, :],
                            in_=hT_re[:, 8 * g : 8 * g + 8, t0 : t0 + 512],
                        )
                    ht_q.append(t)
                def v_group(g):
                    ps = ps_wk.tile([128, 4, HDC], F32, tag="work",
                                   name=f"v{b}{half}{g}")
                    for sub in range(4):
                        for fc in range(FC):
                            nc.tensor.matmul(
                                ps[:, sub, :],
                                ht_q[g][:, fc, sub * 128 : (sub + 1) * 128],
                                w_sb["v"][:, fc, :],
                                start=(fc == 0),
                                stop=(fc == FC - 1),
                            )
                    tt0 = half * 8 + g * 4
                    nc.vector.tensor_add(
                        v_sb[b][:, tt0 : tt0 + 4, :], ps, bv4)

                v_group(0)
                # Q^T / K^T feature-major
                for h in range(HPC):
                    for name, dst, bias in (
                        ("q", qt_sb[b][h], bq_sb),
                        ("k", kt_sb[b][h], bk_sb),
                    ):
                        ps = ps_wk.tile([128, 1024], F32, tag="work",
                                       name=f"qk{b}{half}{h}{name}")
                        for fc in range(FC):
                            lhsT = w_sb[name][:, fc, h * HD : (h + 1) * HD]
                            for n in range(2):
                                nc.tensor.matmul(
                                    ps[:, n * 512 : (n + 1) * 512],
                                    lhsT,
                                    ht_q[n][:, fc, :],
                                    start=(fc == 0),
                                    stop=(fc == FC - 1),
                                )
                        nc.vector.tensor_scalar_add(
                            dst[:, half * 1024 : (half + 1) * 1024],
                            ps, bias[:, h : h + 1],
                        )
                v_group(1)

        def attention(b, qh):
            for h in range(HPC):
                    q0 = qh * 1024
                    av = ps_wk.tile([128, 1024], F32, tag="work",
                                   name=f"av{b}{h}{qh}")
                    pts = []
                    quads = []
                    for tcx in range(TC):
                        ps = ps_sc.tile([128, 1024], F32, tag="sc",
                                       name=f"sc{b}{h}{qh}{tcx}")
                        lhsT = kt_sb[b][h][:, tcx * 128 : (tcx + 1) * 128]
                        for n in range(2):
                            nc.tensor.matmul(
                                ps[:, n * 512 : (n + 1) * 512],
                                lhsT,
                                qt_sb[b][h][:, q0 + n * 512 : q0 + (n + 1) * 512],
                                start=True,
                                stop=True,
                            )
                        pt = pt_pool.tile([128, 1024], BF16, tag="pt",
                                          name=f"pt{b}{h}{qh}{tcx}")
                        nc.scalar.activation(pt, ps, EXP,
                                             bias=neg_shift, scale=scale)
                        pts.append(pt)
                        for n in range(2):
                            nc.tensor.matmul(
                                av[:, n * 512 : (n + 1) * 512],
                                v_sb[b][:, tcx, h * HD : (h + 1) * HD],
                                pt[:, n * 512 : (n + 1) * 512],
                                start=(tcx == 0),
                                stop=(tcx == TC - 1),
                            )
                        if tcx % 4 == 3:
                            pair0 = pair_pool.tile([128, 1024], BF16, tag="pair",
                                                   name=f"p0{b}{h}{qh}{tcx}")
                            nc.vector.tensor_add(pair0, pts[-4], pts[-3])
                            pair1 = pair_pool.tile([128, 1024], BF16, tag="pair",
                                                   name=f"p1{b}{h}{qh}{tcx}")
                            nc.vector.tensor_add(pair1, pts[-2], pts[-1])
                            quad = quad_pool.tile([128, 1024], BF16, tag="quad",
                                                  name=f"q{b}{h}{qh}{tcx}")
                            nc.vector.tensor_add(quad, pair0, pair1)
                            quads.append(quad)
                    den = ps_sc.tile([128, 1024], F32, tag="sc",
                                    name=f"den{b}{h}{qh}")
                    for i in range(4):
                        for n in range(2):
                            nc.tensor.matmul(
                                den[:, n * 512 : (n + 1) * 512],
                                ones,
                                quads[i][:, n * 512 : (n + 1) * 512],
                                start=(i == 0),
                                stop=(i == 3),
                            )
                    recip = den_pool.tile([128, 1024], F32, tag="recip",
                                          name=f"r{b}{h}{qh}")
                    nc.vector.reciprocal_approx_fast(recip, den)
                    nc.vector.tensor_mul(
                        aoT_sb[b][h][:, q0 : q0 + 1024], av, recip)

        def oproj(b, tts):
            for tt in tts:
                pss = [ps_wk.tile([128, 1024], F32, tag="work",
                                 name=f"o{b}{tt}{i}") for i in range(2)]
                for h in range(HPC):
                    lhsT = aoT_sb[b][h][:, tt * 128 : (tt + 1) * 128]
                    for half2 in range(2):
                        for n in range(2):
                            o0 = half2 * 1024 + n * 512
                            nc.tensor.matmul(
                                pss[half2][:, n * 512 : (n + 1) * 512],
                                lhsT,
                                woT_sb[:, h, o0 : o0 + 512],
                                start=(h == 0),
                                stop=(h == HPC - 1),
                            )
                row0 = b * S + tt * 128
                for half2 in range(2):
                    o_tile = o_sb_pool.tile([128, 1024], F32, tag="o",
                                            name=f"ot{b}{tt}{half2}")
                    if half2 == 0:
                        nc.scalar.activation(o_tile, pss[half2], COPY)
                    else:
                        nc.vector.tensor_copy(o_tile, pss[half2])
                    nc.sync.dma_start(
                        out=out[row0 : row0 + 128,
                                half2 * 1024 : (half2 + 1) * 1024],
                        in_=o_tile,
                    )

        qkv(0)
        attention(0, 0)
        attention(0, 1)
        qkv(1)
        oproj(0, range(0, 8))
        attention(1, 0)
        oproj(0, range(8, TC))
        attention(1, 1)
        oproj(1, range(0, 8))
        oproj(1, range(8, TC))


def kernel(hidden_state, Wq, bq, Wk, bk, Wv, bv, Wo, bo):
    bf16 = ml_dtypes.bfloat16
    h2 = np.asarray(hidden_state, dtype=np.float32).reshape(T, H)
    hT = np.ascontiguousarray(h2.T).astype(bf16)

    in_maps = []
    for c in range(N_CORES):
        r0 = c * HDC
        in_maps.append({
            "hT": hT,
            "wqT": np.ascontiguousarray(
                np.asarray(Wq, np.float32)[r0 : r0 + HDC, :].T).astype(bf16),
            "wkT": np.ascontiguousarray(
                np.asarray(Wk, np.float32)[r0 : r0 + HDC, :].T).astype(bf16),
            "wvT": np.ascontiguousarray(
                np.asarray(Wv, np.float32)[r0 : r0 + HDC, :].T).astype(bf16),
            "woT": np.ascontiguousarray(
                np.asarray(Wo, np.float32)[:, r0 : r0 + HDC].T).astype(bf16),
            "bq": np.asarray(bq, np.float32)[r0 : r0 + HDC].copy(),
            "bk": np.asarray(bk, np.float32)[r0 : r0 + HDC].copy(),
            "bv": np.asarray(bv, np.float32)[r0 : r0 + HDC].reshape(1, HDC).copy(),
        })

    if "nc" not in _CACHE:
        _CACHE["nc"] = build_program()
    nc = _CACHE["nc"]
    _CACHE["in_maps"] = in_maps

    res = run_bass_kernel_spmd(nc, in_maps, core_ids=list(range(N_CORES)))
    total = np.zeros((T, H), np.float32)
    for r in res.results:
        total += np.asarray(r["out"], np.float32)
    total += np.asarray(bo, np.float32)[None, :]
    return total.reshape(B, S, H)
